# revision 1
# baseline (speedup 1.0000x reference)
"""Causal multi-head attention block (B=4,S=2048,D=1024,H=16) on 8 trn2 cores.

Sharding: data-parallel over batch (4) x tensor-parallel over head-groups (2).
Core c handles batch c//2, heads [8*(c%2), 8*(c%2)+8). Each core computes its
partial output projection; the host sums the two head-group partials per batch.
"""

import sys

for p in ("/opt/trn_rl_repo", "/root/.axon_site/_ro/trn_rl_repo"):
    if p not in sys.path:
        sys.path.insert(0, p)

import numpy as np
import ml_dtypes

import concourse.bass as bass
import concourse.mybir as mybir
import concourse.tile as tile
from concourse import bacc
from concourse.bass_utils import run_bass_kernel_spmd

FP32 = mybir.dt.float32
BF16 = mybir.dt.bfloat16
AF = mybir.ActivationFunctionType

B, S, D, H = 4, 2048, 1024, 16
DH = 64
N_CORES = 8
HPC = H // 2  # heads per core (head-group tensor parallel = 2)


def build_nc(s=S, d=D, hpc=HPC, dbg=False, reps=1, ablate=()):
    """Build the per-core SPMD program. All 8 cores run this same program."""
    P = 128
    KC = d // P              # feature chunks (contraction for qkv proj)
    NPAIR = hpc // 2         # head pairs
    VC = hpc * DH            # v columns / a columns per core
    QTS = 512                # query tile size
    NQT = s // QTS           # query tiles
    PC = VC // P             # proj contraction chunks (= NPAIR)
    NOUT = d // 512          # out-proj n tiles

    nc = bacc.Bacc("TRN2", target_bir_lowering=False, debug=False,
                   num_devices=N_CORES)

    xb = nc.dram_tensor("xb", [s, d], BF16, kind="ExternalInput")
    wqk = nc.dram_tensor("wqk", [d, 2 * VC], BF16, kind="ExternalInput")
    wv = nc.dram_tensor("wv", [d, VC], BF16, kind="ExternalInput")
    wp = nc.dram_tensor("wp", [VC, d], BF16, kind="ExternalInput")
    bqk = nc.dram_tensor("bqk", [2 * VC], FP32, kind="ExternalInput")
    bv = nc.dram_tensor("bv", [VC], FP32, kind="ExternalInput")
    bph = nc.dram_tensor("bph", [d], FP32, kind="ExternalInput")
    yp = nc.dram_tensor("yp", [s, d], FP32, kind="ExternalOutput")

    rsd = nc.dram_tensor("rsd", [hpc, 1024], FP32)  # recip-sum staging

    if dbg:
        qTo = nc.dram_tensor("qTo", [128, hpc // 2, s], FP32, kind="ExternalOutput")
        kTo = nc.dram_tensor("kTo", [128, hpc // 2, s], FP32, kind="ExternalOutput")
        vo = nc.dram_tensor("vo", [128, s // 128, hpc, DH + 1], FP32, kind="ExternalOutput")
        aTno = nc.dram_tensor("aTno", [128, hpc // 2, s], FP32, kind="ExternalOutput")

    def m_d(dd):
        k = np.arange(P)[:, None]
        q = np.arange(512)[None, :]
        return (k + dd <= q).astype(ml_dtypes.bfloat16)
    gm0_np = np.concatenate([m_d(0), m_d(128), m_d(0), m_d(128)], axis=1)
    gm1_np = np.concatenate([m_d(256), m_d(384), m_d(256), m_d(384)], axis=1)
    gm0_dram = nc.inline_tensor(gm0_np, name="gm0")
    gm1_dram = nc.inline_tensor(gm1_np, name="gm1")

    with tile.TileContext(nc) as tc:
        with (
            tc.tile_pool(name="singles", bufs=1) as singles,
            tc.tile_pool(name="xt", bufs=12) as xt_pool,
            tc.tile_pool(name="probs", bufs=2) as probs_pool,
            tc.tile_pool(name="norm", bufs=4) as norm_pool,
            tc.tile_pool(name="ysb", bufs=2) as y_pool,
            tc.tile_pool(name="mm512", bufs=2, space="PSUM") as mm_ps,
            tc.tile_pool(name="scps", bufs=1, space="PSUM") as sc_ps,
            tc.tile_pool(name="atps", bufs=1, space="PSUM") as at_ps,
        ):
            # ---- persistent SBUF state ----
            wqk_sb = singles.tile([P, KC, 2 * VC], BF16)
            wv_sb = singles.tile([P, KC, VC], BF16)
            wp_sb = singles.tile([P, PC, d], BF16)
            bqk_sb = singles.tile([P, 2 * VC // P], FP32)
            bv_rep = singles.tile([P, VC], FP32)
            bp_rep = singles.tile([P, d], FP32)
            gm0_sb = singles.tile([P, 2048], BF16)
            gm1_sb = singles.tile([P, 2048], BF16)
            qT = singles.tile([P, NPAIR, s], BF16)
            kT = singles.tile([P, NPAIR, s], BF16)
            v_sb = singles.tile([P, s // P, hpc, DH + 1], BF16)
            aTn = singles.tile([P, NPAIR, s], BF16)

            # ---- constant / weight loads ----
            nc.sync.dma_start(gm0_sb[:], gm0_dram[:])
            nc.sync.dma_start(gm1_sb[:], gm1_dram[:])
            nc.sync.dma_start(out=wqk_sb[:],
                              in_=wqk.rearrange("(c p) n -> p c n", p=P))
            nc.sync.dma_start(out=wv_sb[:],
                              in_=wv.rearrange("(c p) n -> p c n", p=P))
            nc.sync.dma_start(out=wp_sb[:],
                              in_=wp.rearrange("(c p) n -> p c n", p=P))
            nc.sync.dma_start(out=bqk_sb[:],
                              in_=bqk.rearrange("(ct p) -> p ct", p=P))
            nc.sync.dma_start(out=bv_rep[:], in_=bv.rearrange("(a b) -> a b", a=1).to_broadcast((P, VC)))
            nc.sync.dma_start(out=bp_rep[:], in_=bph.rearrange("(a b) -> a b", a=1).to_broadcast((P, d)))
            # ones column of v' (fused row-sum trick)
            nc.vector.memset(v_sb[:, :, :, DH], 1.0)

            for rep in range(reps):
              XB = 2 if NQT % 2 == 0 else 1
              xts2 = {}
              for tt in range(NQT):
                  ts0 = tt * QTS
                  # ---- x^T tiles (feature-major); XB token tiles per DMA ----
                  if tt % XB == 0:
                      xts2 = {}
                      for kc in range(KC):
                          xt2 = xt_pool.tile([P, XB * QTS], BF16)
                          nc.sync.dma_start(
                              out=xt2[:],
                              in_=xb[ts0:ts0 + XB * QTS,
                                     kc * P:(kc + 1) * P],
                              transpose=True)
                          xts2[kc] = xt2
                  off = (tt % XB) * QTS
                  xts = [xts2[kc][:, off:off + QTS] for kc in range(KC)]

                  # ---- q^T / k^T projection (feature-major out) ----
                  for ct in range(2 * VC // P):
                      ps = mm_ps.tile([P, QTS], FP32, tag="mm512")
                      for kc in range(KC):
                          nc.tensor.matmul(
                              ps[:], wqk_sb[:, kc, ct * P:(ct + 1) * P],
                              xts[kc][:], start=(kc == 0), stop=(kc == KC - 1))
                      pair, is_k = ct % NPAIR, ct // NPAIR
                      dst = (kT if is_k else qT)[:, pair, ts0:ts0 + QTS]
                      nc.vector.tensor_scalar_add(dst, ps[:], bqk_sb[:, ct:ct + 1])

                  # ---- v projection (token-major out) ----
                  for sub in range(QTS // P):
                      ps = mm_ps.tile([P, VC], FP32, tag="mm512")
                      for kc in range(KC):
                          nc.tensor.matmul(
                              ps[:], xts[kc][:, sub * P:(sub + 1) * P],
                              wv_sb[:, kc, :], start=(kc == 0),
                              stop=(kc == KC - 1))
                      vt = tt * (QTS // P) + sub
                      nc.vector.tensor_add(
                          v_sb[:, vt, :, 0:DH],
                          ps[:].rearrange("p (h e) -> p h e", e=DH),
                          bv_rep[:].rearrange("p (h e) -> p h e", e=DH))

                  # ---- attention for query tile tt, all head pairs ----
                  j = tt
                  nkt = 4 * (j + 1)  # causal: k tiles 0 .. nkt-1
                  for pair in range(NPAIR):
                      at_A = at_ps.tile([P, QTS], FP32, tag="atA")
                      at_B = at_ps.tile([P, QTS], FP32, tag="atB")
                      for grp in range(nkt // 2):
                          sc = sc_ps.tile([P, 2048], FP32, tag="sc")
                          for i in range(2):
                              kt = 2 * grp + i
                              nc.tensor.matmul(
                                  sc[:, i * 512:(i + 1) * 512],
                                  kT[0:DH, pair, kt * P:(kt + 1) * P],
                                  qT[0:DH, pair, ts0:ts0 + QTS],
                                  start=True, stop=True)
                              nc.tensor.matmul(
                                  sc[:, 1024 + i * 512:1024 + (i + 1) * 512],
                                  kT[DH:P, pair, kt * P:(kt + 1) * P],
                                  qT[DH:P, pair, ts0:ts0 + QTS],
                                  start=True, stop=True)
                          pr = probs_pool.tile([P, 2048], BF16)
                          nc.scalar.activation(pr[:], sc[:], AF.Exp,
                                               scale=1.0 / np.sqrt(DH))
                          # causal mask: one mul with precomputed group mask
                          if grp == 2 * j:
                              nc.vector.tensor_mul(pr[:], pr[:], gm0_sb[:])
                          elif grp == 2 * j + 1:
                              nc.vector.tensor_mul(pr[:], pr[:], gm1_sb[:])
                          for i in range(2):
                              kt = 2 * grp + i
                              for h01, at in ((0, at_A), (1, at_B)):
                                  nc.tensor.matmul(
                                      at[0:DH + 1, :],
                                      v_sb[:, kt, 2 * pair + h01, :],
                                      pr[:, h01 * 1024 + i * 512:
                                         h01 * 1024 + (i + 1) * 512],
                                      start=(kt == 0), stop=(kt == nkt - 1))
                      # ---- normalize: a^T / rowsum, store to aTn ----
                      # batch both heads' recip-sums into one staging DMA +
                      # one broadcast DMA per (pair, qtile)
                      rs = norm_pool.tile([1, 2 * QTS], FP32, tag="rs")
                      nc.vector.reciprocal(rs[:, 0:QTS], at_A[DH:DH + 1, :])
                      nc.vector.reciprocal(rs[:, QTS:], at_B[DH:DH + 1, :])
                      nc.sync.dma_start(out=rsd[2 * pair:2 * pair + 1, :],
                                        in_=rs[:])
                      rc = norm_pool.tile([P, 2 * QTS], FP32, tag="rc")
                      nc.sync.dma_start(
                          out=rc[:],
                          in_=rsd[2 * pair:2 * pair + 1, :]
                          .to_broadcast((P, 2 * QTS)))
                      nc.vector.tensor_mul(
                          aTn[0:DH, pair, ts0:ts0 + QTS],
                          at_A[0:DH, :], rc[0:DH, 0:QTS])
                      tmp = norm_pool.tile([DH, QTS], BF16, tag="tmpB")
                      nc.vector.tensor_mul(tmp[:], at_B[0:DH, :],
                                           rc[0:DH, QTS:])
                      nc.sync.dma_start(
                          out=aTn[DH:P, pair, ts0:ts0 + QTS],
                          in_=tmp[:])

                  # ---- partial out-projection for this token tile ----
                  for sub in range(QTS // P):
                      ysb = y_pool.tile([P, d], FP32)
                      t0 = ts0 + sub * P
                      for n in range(NOUT):
                          ps = mm_ps.tile([P, 512], FP32, tag="mm512")
                          for pc in range(PC):
                              nc.tensor.matmul(
                                  ps[:], aTn[:, pc, t0:t0 + P],
                                  wp_sb[:, pc, n * 512:(n + 1) * 512],
                                  start=(pc == 0), stop=(pc == PC - 1))
                          nc.vector.tensor_add(ysb[:, n * 512:(n + 1) * 512],
                                               ps[:],
                                               bp_rep[:, n * 512:(n + 1) * 512])
                      if "yout" not in ablate:
                        nc.sync.dma_start(out=yp[t0:t0 + P, :], in_=ysb[:])

            if dbg:
                for name, src, dst in (("qT", qT, qTo), ("kT", kT, kTo),
                                       ("v", v_sb, vo), ("aTn", aTn, aTno)):
                    t = singles.tile(list(src.shape), FP32, tag="d" + name)
                    nc.vector.tensor_copy(t[:], src[:])
                    nc.sync.dma_start(out=dst[:], in_=t[:])

    nc.compile()
    return nc


_NC_CACHE = {}


def _get_nc():
    if "nc" not in _NC_CACHE:
        _NC_CACHE["nc"] = build_nc()
    return _NC_CACHE["nc"]


def make_in_maps(x, w_attn, b_attn, w_proj, b_proj):
    """Host-side sharding: batch c//2, head-group c%2."""
    VC = HPC * DH  # 512
    wq, wk, wv = w_attn[:, :D], w_attn[:, D:2 * D], w_attn[:, 2 * D:]
    bq, bk, bv = b_attn[:D], b_attn[D:2 * D], b_attn[2 * D:]
    in_maps = []
    for c in range(N_CORES):
        b, g = c // 2, c % 2
        sl = slice(g * VC, (g + 1) * VC)
        bf = ml_dtypes.bfloat16
        in_maps.append({
            "xb": np.ascontiguousarray(x[b].astype(bf)),
            "wqk": np.ascontiguousarray(
                np.concatenate([wq[:, sl], wk[:, sl]], axis=1).astype(bf)),
            "wv": np.ascontiguousarray(wv[:, sl].astype(bf)),
            "wp": np.ascontiguousarray(
                w_proj[g * VC:(g + 1) * VC, :].astype(bf)),
            "bqk": np.ascontiguousarray(
                np.concatenate([bq[sl], bk[sl]])),
            "bv": np.ascontiguousarray(bv[sl]),
            "bph": np.ascontiguousarray(b_proj * 0.5),
        })
    return in_maps


def kernel(x, w_attn, b_attn, w_proj, b_proj):
    x = np.asarray(x, dtype=np.float32)
    w_attn = np.asarray(w_attn, dtype=np.float32)
    b_attn = np.asarray(b_attn, dtype=np.float32)
    w_proj = np.asarray(w_proj, dtype=np.float32)
    b_proj = np.asarray(b_proj, dtype=np.float32)

    nc = _get_nc()
    in_maps = make_in_maps(x, w_attn, b_attn, w_proj, b_proj)
    res = run_bass_kernel_spmd(nc, in_maps, core_ids=list(range(N_CORES)))
    out = np.empty((B, S, D), dtype=np.float32)
    for b in range(B):
        out[b] = res.results[2 * b]["yp"] + res.results[2 * b + 1]["yp"]
    return out



# revision 2
# speedup vs baseline: 1.3985x; 1.3985x over previous
"""Causal MHA block (B=4,S=2048,D=1024,H=16) on 8 trn2 cores.

Sharding: DP over batch (4) x TP over head-groups (2); host sums the two
partial projections per batch.

Three hardware loops (qtile, head-pair, k-group); projection phases are
unrolled (loop machinery would cost more than their bodies). The k-group
loop covers all causal groups with a mask selected by a clamped register
index from [ones, gm0, gm1]. Normalization is on-chip (ones column rowsums,
reciprocal row broadcast via K=1 matmuls).
"""

import sys

for p in ("/opt/trn_rl_repo", "/root/.axon_site/_ro/trn_rl_repo"):
    if p not in sys.path:
        sys.path.insert(0, p)

import numpy as np
import ml_dtypes

import concourse.bass as bass
import concourse.mybir as mybir
import concourse.tile as tile
from concourse import bacc
from concourse.bass_utils import run_bass_kernel_spmd

FP32 = mybir.dt.float32
BF16 = mybir.dt.bfloat16
AF = mybir.ActivationFunctionType
ALU = mybir.AluOpType
ds = bass.ds

B, S, D, H = 4, 2048, 1024, 16
DH = 64
N_CORES = 8
HPC = 8
VC = HPC * DH    # 512
P = 128
KC = D // P      # 8
QTS = 512
NQT = S // QTS   # 4
NPAIR = HPC // 2  # 4


def _mask(dd):
    k = np.arange(P)[:, None]
    q = np.arange(QTS)[None, :]
    return (k + dd <= q).astype(ml_dtypes.bfloat16)


def build_nc(reps=1):
    nc = bacc.Bacc("TRN2", target_bir_lowering=False, debug=False,
                   num_devices=N_CORES)

    xq = nc.dram_tensor("xq", [NQT, P, KC * QTS], BF16, kind="ExternalInput")
    wqk = nc.dram_tensor("wqk", [P, KC, 2 * VC], BF16, kind="ExternalInput")
    wv = nc.dram_tensor("wv", [P, KC, VC], BF16, kind="ExternalInput")
    wp = nc.dram_tensor("wp", [P, NPAIR, D], BF16, kind="ExternalInput")
    bqk = nc.dram_tensor("bqk", [P, 8], FP32, kind="ExternalInput")
    bv = nc.dram_tensor("bv", [P, VC], FP32, kind="ExternalInput")
    bpT = nc.dram_tensor("bpT", [P, 8], FP32, kind="ExternalInput")
    ypT = nc.dram_tensor("ypT", [NQT, P, 8 * QTS], BF16,
                         kind="ExternalOutput")

    ones_np = np.ones((P, 2048), ml_dtypes.bfloat16)
    gm0_np = np.concatenate([_mask(0), _mask(128), _mask(0), _mask(128)],
                            axis=1)
    gm1_np = np.concatenate([_mask(256), _mask(384), _mask(256), _mask(384)],
                            axis=1)
    gm_np = np.stack([ones_np, gm0_np, gm1_np], axis=1)  # [128, 3, 2048]
    gm_d = nc.inline_tensor(gm_np.reshape(P, 3 * 2048), name="gm")
    zrow_d = nc.inline_tensor(np.zeros((1, 512), np.float32), name="zrow")
    ones64_d = nc.inline_tensor(np.ones((1, 64), np.float32), name="ones64")

    with tile.TileContext(nc) as tc:
        with (
            tc.tile_pool(name="singles", bufs=1) as sg,
            tc.tile_pool(name="xt", bufs=1) as xt_pool,
            tc.tile_pool(name="yt", bufs=1) as yt_pool,
            tc.tile_pool(name="pr", bufs=1) as pr_pool,
            tc.tile_pool(name="kg", bufs=1) as kg_pool,
            tc.tile_pool(name="bigp", bufs=1, space="PSUM") as bigp,
        ):
            wqk_sb = sg.tile([P, KC, 2 * VC], BF16)
            wv_sb = sg.tile([P, KC, VC], BF16)
            wp_sb = sg.tile([P, NPAIR, D], BF16)
            bqk_sb = sg.tile([P, 8], FP32)
            bv_sb = sg.tile([P, VC], FP32)
            bpT_sb = sg.tile([P, 8], FP32)
            gm_sb = sg.tile([P, 3, 2048], BF16)
            zrow = sg.tile([1, 512], FP32)
            ones64 = sg.tile([1, 64], FP32)
            qT = sg.tile([P, NPAIR, S], BF16)
            kT = sg.tile([P, NPAIR, S], BF16)
            v_sb = sg.tile([P, S // P, HPC, DH + 1], BF16)
            aTn = sg.tile([P, NPAIR, S], BF16)
            rr = sg.tile([1, 1024], FP32)

            nc.sync.dma_start(out=wqk_sb[:], in_=wqk[:])
            nc.sync.dma_start(out=wv_sb[:], in_=wv[:])
            nc.sync.dma_start(out=wp_sb[:], in_=wp[:])
            nc.sync.dma_start(out=bqk_sb[:], in_=bqk[:])
            nc.sync.dma_start(out=bv_sb[:], in_=bv[:])
            nc.sync.dma_start(out=bpT_sb[:], in_=bpT[:])
            nc.sync.dma_start(out=gm_sb[:].rearrange("p a b -> p (a b)"),
                              in_=gm_d[:])
            nc.sync.dma_start(out=zrow[:], in_=zrow_d[:])
            nc.sync.dma_start(out=ones64[:], in_=ones64_d[:])
            nc.vector.memset(v_sb[:, :, :, DH], 1.0)

            for rep in range(reps):
                with tc.For_i(0, NQT, 1) as i:
                    # ---- x tile: one contiguous-per-partition DMA ----
                    xt = xt_pool.tile([P, KC, QTS], BF16, tag="xt")
                    nc.sync.dma_start(
                        out=xt[:].rearrange("p c t -> p (c t)"),
                        in_=xq[ds(i, 1), :, :].rearrange("o p n -> (o p) n"))

                    # ---- qk projection: accumulate over dynamic kc ----
                    big = bigp.tile([P, 4096], FP32, tag="big")
                    for ct in range(8):
                        nc.tensor.matmul(big[:, ct * 512:(ct + 1) * 512],
                                         zrow[0:1, 0:P], zrow[0:1, :],
                                         start=True, stop=True,
                                         skip_group_check=True)
                    with tc.For_i(0, KC, 1) as kc:
                        wcur = kg_pool.tile([P, 2 * VC], BF16, tag="wcur")
                        nc.vector.tensor_copy(
                            wcur[:].rearrange("p (o n) -> p o n", o=1),
                            wqk_sb[:, ds(kc, 1), :])
                        xcur = kg_pool.tile([P, QTS], BF16, tag="xcur")
                        nc.vector.tensor_copy(
                            xcur[:].rearrange("p (o n) -> p o n", o=1),
                            xt[:, ds(kc, 1), :])
                        for ct in range(8):
                            nc.tensor.matmul(
                                big[:, ct * 512:(ct + 1) * 512],
                                wcur[:, ct * P:(ct + 1) * P], xcur[:],
                                start=False, stop=False,
                                skip_group_check=True)
                    for ct in range(8):
                        dst = (qT if ct < 4 else kT)[:, ct % 4,
                                                     ds(i * QTS, QTS)]
                        nc.vector.tensor_scalar_add(
                            dst, big[:, ct * 512:(ct + 1) * 512],
                            bqk_sb[:, ct:ct + 1])

                    # ---- v projection: accumulate over dynamic kc ----
                    big = bigp.tile([P, 4096], FP32, tag="big")
                    for sub in range(4):
                        nc.tensor.matmul(big[:, sub * 512:(sub + 1) * 512],
                                         zrow[0:1, 0:P], zrow[0:1, :],
                                         start=True, stop=True,
                                         skip_group_check=True)
                    with tc.For_i(0, KC, 1) as kc:
                        xcur = kg_pool.tile([P, QTS], BF16, tag="xcur")
                        nc.vector.tensor_copy(
                            xcur[:].rearrange("p (o n) -> p o n", o=1),
                            xt[:, ds(kc, 1), :])
                        wvcur = kg_pool.tile([P, VC], BF16, tag="wvcur")
                        nc.vector.tensor_copy(
                            wvcur[:].rearrange("p (o n) -> p o n", o=1),
                            wv_sb[:, ds(kc, 1), :])
                        for sub in range(4):
                            nc.tensor.matmul(
                                big[:, sub * 512:(sub + 1) * 512],
                                xcur[:, sub * P:(sub + 1) * P], wvcur[:],
                                start=False, stop=False,
                                skip_group_check=True)
                    for sub in range(4):
                        nc.vector.tensor_add(
                            v_sb[:, ds(i * 4 + sub, 1), :, 0:DH],
                            big[:, sub * 512:(sub + 1) * 512].rearrange(
                                "p (o h e) -> p o h e", o=1, e=DH),
                            bv_sb[:].rearrange("p (o h e) -> p o h e",
                                               o=1, e=DH))

                    # ---- attention: dynamic pair loop, merged group loop --
                    with tc.For_i(0, NPAIR, 1) as pair:
                        big = bigp.tile([P, 4096], FP32, tag="big")
                        qg = kg_pool.tile([P, QTS], BF16, tag="qg")
                        nc.vector.tensor_copy(
                            qg[:].rearrange("p (o n) -> p o n", o=1),
                            qT[:, ds(pair, 1), ds(i * QTS, QTS)])
                        for h in range(2):  # clear at2 region
                            nc.tensor.matmul(
                                big[0:DH + 1,
                                    2048 + h * 512:2048 + (h + 1) * 512],
                                zrow[0:1, 0:DH + 1], zrow[0:1, :],
                                start=True, stop=True, skip_group_check=True)
                        with tc.For_i(0, 2 * i + 2, 1) as g:
                            ktg = kg_pool.tile([P, 256], BF16, tag="ktg")
                            vg = kg_pool.tile([P, 2, 2, DH + 1], BF16,
                                              tag="vg")
                            nc.vector.tensor_copy(
                                ktg[:].rearrange("p (o n) -> p o n", o=1),
                                kT[:, ds(pair, 1), ds(g * 256, 256)])
                            nc.vector.tensor_copy(
                                vg[:], v_sb[:, ds(g * 2, 2),
                                            ds(pair * 2, 2), :])
                            for h in range(2):
                                for t in range(2):
                                    nc.tensor.matmul(
                                        big[:, (2 * h + t) * 512:
                                            (2 * h + t + 1) * 512],
                                        ktg[h * DH:(h + 1) * DH,
                                            t * 128:(t + 1) * 128],
                                        qg[h * DH:(h + 1) * DH, :],
                                        start=True, stop=True)
                            pr = pr_pool.tile([P, 2048], BF16, tag="pr")
                            nc.scalar.activation(pr[:], big[:, 0:2048],
                                                 AF.Exp,
                                                 scale=1.0 / np.sqrt(DH))
                            # mask index: max(g-2i, -1)+1 -> 0 ones, 1 gm0,
                            # 2 gm1
                            rv = nc.vector.alloc_register()
                            nc.vector.reg_alu(rv, g - 2 * i, -1, ALU.max)
                            nc.vector.reg_add(rv, rv, 1)
                            midx = nc.snap(rv, min_val=0, max_val=2)
                            nc.vector.tensor_mul(
                                pr[:].rearrange("p (o n) -> p o n", o=1),
                                pr[:].rearrange("p (o n) -> p o n", o=1),
                                gm_sb[:, ds(midx, 1), :])
                            for h in range(2):
                                for t in range(2):
                                    nc.tensor.matmul(
                                        big[0:DH + 1, 2048 + h * 512:
                                            2048 + (h + 1) * 512],
                                        vg[:, t, h, :],
                                        pr[:, (2 * h + t) * 512:
                                           (2 * h + t + 1) * 512],
                                        start=False, stop=False,
                                        skip_group_check=True)
                        # ---- normalize ----
                        nc.vector.reciprocal(rr[0:1, :],
                                             big[DH:DH + 1, 2048:3072])
                        nc.tensor.matmul(big[0:DH, 3072:3584], ones64[0:1, :],
                                         rr[0:1, 0:512], start=True,
                                         stop=True)
                        nc.tensor.matmul(big[0:DH, 3584:4096], ones64[0:1, :],
                                         rr[0:1, 512:1024], start=True,
                                         stop=True)
                        rcs = kg_pool.tile([DH, 1024], FP32, tag="rcs")
                        nc.vector.tensor_copy(rcs[:], big[0:DH, 3072:4096])
                        aTg = kg_pool.tile([P, QTS], BF16, tag="aTg")
                        nc.vector.tensor_mul(aTg[0:DH, :],
                                             big[0:DH, 2048:2560],
                                             rcs[:, 0:512])
                        nc.vector.tensor_mul(aTg[DH:P, :],
                                             big[0:DH, 2560:3072],
                                             rcs[:, 512:1024])
                        nc.vector.tensor_copy(
                            aTn[:, ds(pair, 1), ds(i * QTS, QTS)],
                            aTg[:].rearrange("p (o n) -> p o n", o=1))

                    # ---- out projection: accumulate over dynamic pc ----
                    yt = yt_pool.tile([P, 8, QTS], BF16, tag="yt")
                    big = bigp.tile([P, 4096], FP32, tag="big")
                    for db in range(8):
                        nc.tensor.matmul(big[:, db * 512:(db + 1) * 512],
                                         zrow[0:1, 0:P], zrow[0:1, :],
                                         start=True, stop=True,
                                         skip_group_check=True)
                    with tc.For_i(0, NPAIR, 1) as pc:
                        wpcur = kg_pool.tile([P, D], BF16, tag="wpcur")
                        nc.vector.tensor_copy(
                            wpcur[:].rearrange("p (o n) -> p o n", o=1),
                            wp_sb[:, ds(pc, 1), :])
                        acur = kg_pool.tile([P, QTS], BF16, tag="acur")
                        nc.vector.tensor_copy(
                            acur[:].rearrange("p (o n) -> p o n", o=1),
                            aTn[:, ds(pc, 1), ds(i * QTS, QTS)])
                        for db in range(8):
                            nc.tensor.matmul(
                                big[:, db * 512:(db + 1) * 512],
                                wpcur[:, db * P:(db + 1) * P], acur[:],
                                start=False, stop=False,
                                skip_group_check=True)
                    for db in range(8):
                        nc.vector.tensor_scalar_add(
                            yt[:, db, :], big[:, db * 512:(db + 1) * 512],
                            bpT_sb[:, db:db + 1])
                    nc.sync.dma_start(
                        out=ypT[ds(i, 1), :, :].rearrange("o p n -> (o p) n"),
                        in_=yt[:].rearrange("p c t -> p (c t)"))

    nc.compile()
    return nc


_NC_CACHE = {}


def _get_nc():
    if "nc" not in _NC_CACHE:
        _NC_CACHE["nc"] = build_nc()
    return _NC_CACHE["nc"]


def make_in_maps(x, w_attn, b_attn, w_proj, b_proj):
    """Host-side sharding + layout prep: batch c//2, head-group c%2."""
    bf = ml_dtypes.bfloat16
    wq, wk, wvf = w_attn[:, :D], w_attn[:, D:2 * D], w_attn[:, 2 * D:]
    bq, bk, bvf = b_attn[:D], b_attn[D:2 * D], b_attn[2 * D:]
    in_maps = []
    for c in range(N_CORES):
        b, g = c // 2, c % 2
        sl = slice(g * VC, (g + 1) * VC)

        xT = np.ascontiguousarray(x[b].T.astype(bf))          # [1024, 2048]
        # xq[i, p, c*512+t] = xT[c*128+p, i*512+t]
        xq_np = np.ascontiguousarray(
            xT.reshape(KC, P, NQT, QTS).transpose(2, 1, 0, 3)
            .reshape(NQT, P, KC * QTS))

        def wlay(w):
            return np.ascontiguousarray(
                w.reshape(KC, P, w.shape[1]).transpose(1, 0, 2).astype(bf))

        wqk_np = wlay(np.concatenate([wq[:, sl], wk[:, sl]], axis=1))
        wv_np = wlay(wvf[:, sl])
        wp_np = np.ascontiguousarray(
            w_proj[g * VC:(g + 1) * VC, :].reshape(NPAIR, P, D)
            .transpose(1, 0, 2).astype(bf))

        bqk_np = np.ascontiguousarray(
            np.concatenate([bq[sl], bk[sl]]).reshape(8, P).T
            .astype(np.float32))
        bv_np = np.ascontiguousarray(
            np.broadcast_to(bvf[sl][None, :], (P, VC)).astype(np.float32))
        bpT_np = np.ascontiguousarray(
            (b_proj * 0.5).reshape(8, P).T.astype(np.float32))

        in_maps.append({"xq": xq_np, "wqk": wqk_np, "wv": wv_np, "wp": wp_np,
                        "bqk": bqk_np, "bv": bv_np, "bpT": bpT_np})
    return in_maps


def kernel(x, w_attn, b_attn, w_proj, b_proj):
    x = np.asarray(x, dtype=np.float32)
    w_attn = np.asarray(w_attn, dtype=np.float32)
    b_attn = np.asarray(b_attn, dtype=np.float32)
    w_proj = np.asarray(w_proj, dtype=np.float32)
    b_proj = np.asarray(b_proj, dtype=np.float32)

    nc = _get_nc()
    in_maps = make_in_maps(x, w_attn, b_attn, w_proj, b_proj)
    res = run_bass_kernel_spmd(nc, in_maps, core_ids=list(range(N_CORES)))
    out = np.empty((B, S, D), dtype=np.float32)
    for b in range(B):
        parts = []
        for c in (2 * b, 2 * b + 1):
            yp = np.asarray(res.results[c]["ypT"],
                            dtype=np.float32).reshape(NQT, P, 8, QTS)
            parts.append(yp.transpose(0, 3, 2, 1).reshape(S, D))
        out[b] = parts[0] + parts[1]
    return out


# revision 3
# speedup vs baseline: 1.4465x; 1.0343x over previous
"""Causal MHA block (B=4,S=2048,D=1024,H=16) on 8 trn2 cores.

Sharding: DP over batch (4) x TP over head-groups (2); host sums the two
partial projections per batch.

Three hardware loops (qtile, head-pair, k-group); projection phases are
unrolled (loop machinery would cost more than their bodies). The k-group
loop covers all causal groups with a mask selected by a clamped register
index from [ones, gm0, gm1]. Normalization is on-chip (ones column rowsums,
reciprocal row broadcast via K=1 matmuls).
"""

import sys

for p in ("/opt/trn_rl_repo", "/root/.axon_site/_ro/trn_rl_repo"):
    if p not in sys.path:
        sys.path.insert(0, p)

import numpy as np
import ml_dtypes

import concourse.bass as bass
import concourse.mybir as mybir
import concourse.tile as tile
from concourse import bacc
from concourse.bass_utils import run_bass_kernel_spmd

FP32 = mybir.dt.float32
BF16 = mybir.dt.bfloat16
AF = mybir.ActivationFunctionType
ALU = mybir.AluOpType
ds = bass.ds

B, S, D, H = 4, 2048, 1024, 16
DH = 64
N_CORES = 8
HPC = 8
VC = HPC * DH    # 512
P = 128
KC = D // P      # 8
QTS = 512
NQT = S // QTS   # 4
NPAIR = HPC // 2  # 4


def _mask(dd):
    k = np.arange(P)[:, None]
    q = np.arange(QTS)[None, :]
    return (k + dd <= q).astype(ml_dtypes.bfloat16)


def build_nc(reps=1):
    nc = bacc.Bacc("TRN2", target_bir_lowering=False, debug=False,
                   num_devices=N_CORES)

    xq = nc.dram_tensor("xq", [NQT, P, KC * QTS], BF16, kind="ExternalInput")
    wqk = nc.dram_tensor("wqk", [P, KC, 2 * VC], BF16, kind="ExternalInput")
    wv = nc.dram_tensor("wv", [P, KC, VC], BF16, kind="ExternalInput")
    wp = nc.dram_tensor("wp", [P, NPAIR, D], BF16, kind="ExternalInput")
    bqk = nc.dram_tensor("bqk", [P, 8], FP32, kind="ExternalInput")
    bv = nc.dram_tensor("bv", [P, VC], FP32, kind="ExternalInput")
    bpT = nc.dram_tensor("bpT", [P, 8], FP32, kind="ExternalInput")
    ypT = nc.dram_tensor("ypT", [NQT, P, 8 * QTS], BF16,
                         kind="ExternalOutput")

    zs_np = np.zeros((P, 2048), np.float32)
    gm0_np = np.concatenate([_mask(0), _mask(128), _mask(0), _mask(128)],
                            axis=1).astype(np.float32)
    gm1_np = np.concatenate([_mask(256), _mask(384), _mask(256), _mask(384)],
                            axis=1).astype(np.float32)
    gm_np = np.stack([zs_np, (gm0_np - 1.0) * 1e9, (gm1_np - 1.0) * 1e9],
                     axis=1)  # [128, 3, 2048] additive
    gm_d = nc.inline_tensor(gm_np.reshape(P, 3 * 2048), name="gm")
    zrow_d = nc.inline_tensor(np.zeros((1, 512), np.float32), name="zrow")
    ones64_d = nc.inline_tensor(np.ones((1, 64), np.float32), name="ones64")

    with tile.TileContext(nc) as tc:
        with (
            tc.tile_pool(name="singles", bufs=1) as sg,
            tc.tile_pool(name="xt", bufs=1) as xt_pool,
            tc.tile_pool(name="yt", bufs=1) as yt_pool,
            tc.tile_pool(name="pr", bufs=1) as pr_pool,
            tc.tile_pool(name="kg", bufs=1) as kg_pool,
            tc.tile_pool(name="bigp", bufs=1, space="PSUM") as bigp,
        ):
            wqk_sb = sg.tile([P, KC, 2 * VC], BF16)
            wv_sb = sg.tile([P, KC, VC], BF16)
            wp_sb = sg.tile([P, NPAIR, D], BF16)
            bqk_sb = sg.tile([P, 8], FP32)
            bv_sb = sg.tile([P, VC], FP32)
            bpT_sb = sg.tile([P, 8], FP32)
            gm_sb = sg.tile([P, 3, 2048], FP32)
            zrow = sg.tile([1, 512], FP32)
            ones64 = sg.tile([1, 64], FP32)
            qT = sg.tile([P, NPAIR, S], BF16)
            kT = sg.tile([P, NPAIR, S], BF16)
            v_sb = sg.tile([P, S // P, HPC, DH + 1], BF16)
            aTn = sg.tile([P, NPAIR, S], BF16)
            rr = sg.tile([1, 1024], FP32)

            nc.sync.dma_start(out=wqk_sb[:], in_=wqk[:])
            nc.sync.dma_start(out=wv_sb[:], in_=wv[:])
            nc.sync.dma_start(out=wp_sb[:], in_=wp[:])
            nc.sync.dma_start(out=bqk_sb[:], in_=bqk[:])
            nc.sync.dma_start(out=bv_sb[:], in_=bv[:])
            nc.sync.dma_start(out=bpT_sb[:], in_=bpT[:])
            nc.sync.dma_start(out=gm_sb[:].rearrange("p a b -> p (a b)"),
                              in_=gm_d[:])
            nc.sync.dma_start(out=zrow[:], in_=zrow_d[:])
            nc.sync.dma_start(out=ones64[:], in_=ones64_d[:])
            nc.vector.memset(v_sb[:, :, :, DH], 1.0)

            for rep in range(reps):
                with tc.For_i(0, NQT, 1) as i:
                    # ---- x tile: one contiguous-per-partition DMA ----
                    xt = xt_pool.tile([P, KC, QTS], BF16, tag="xt")
                    nc.sync.dma_start(
                        out=xt[:].rearrange("p c t -> p (c t)"),
                        in_=xq[ds(i, 1), :, :].rearrange("o p n -> (o p) n"))

                    # ---- qk projection: accumulate over dynamic kc ----
                    big = bigp.tile([P, 4096], FP32, tag="big")
                    for ct in range(8):
                        nc.tensor.matmul(big[:, ct * 512:(ct + 1) * 512],
                                         zrow[0:1, 0:P], zrow[0:1, :],
                                         start=True, stop=True,
                                         skip_group_check=True)
                    with tc.For_i(0, KC, 1) as kc:
                        wcur = kg_pool.tile([P, 2 * VC], BF16, tag="wcur")
                        nc.vector.tensor_copy(
                            wcur[:].rearrange("p (o n) -> p o n", o=1),
                            wqk_sb[:, ds(kc, 1), :])
                        xcur = kg_pool.tile([P, QTS], BF16, tag="xcur")
                        nc.vector.tensor_copy(
                            xcur[:].rearrange("p (o n) -> p o n", o=1),
                            xt[:, ds(kc, 1), :])
                        for ct in range(8):
                            nc.tensor.matmul(
                                big[:, ct * 512:(ct + 1) * 512],
                                wcur[:, ct * P:(ct + 1) * P], xcur[:],
                                start=False, stop=False,
                                skip_group_check=True)
                    for ct in range(8):
                        dst = (qT if ct < 4 else kT)[:, ct % 4,
                                                     ds(i * QTS, QTS)]
                        nc.vector.tensor_scalar_add(
                            dst, big[:, ct * 512:(ct + 1) * 512],
                            bqk_sb[:, ct:ct + 1])

                    # ---- v projection: accumulate over dynamic kc ----
                    big = bigp.tile([P, 4096], FP32, tag="big")
                    for sub in range(4):
                        nc.tensor.matmul(big[:, sub * 512:(sub + 1) * 512],
                                         zrow[0:1, 0:P], zrow[0:1, :],
                                         start=True, stop=True,
                                         skip_group_check=True)
                    with tc.For_i(0, KC, 1) as kc:
                        xcur = kg_pool.tile([P, QTS], BF16, tag="xcur")
                        nc.vector.tensor_copy(
                            xcur[:].rearrange("p (o n) -> p o n", o=1),
                            xt[:, ds(kc, 1), :])
                        wvcur = kg_pool.tile([P, VC], BF16, tag="wvcur")
                        nc.vector.tensor_copy(
                            wvcur[:].rearrange("p (o n) -> p o n", o=1),
                            wv_sb[:, ds(kc, 1), :])
                        for sub in range(4):
                            nc.tensor.matmul(
                                big[:, sub * 512:(sub + 1) * 512],
                                xcur[:, sub * P:(sub + 1) * P], wvcur[:],
                                start=False, stop=False,
                                skip_group_check=True)
                    for sub in range(4):
                        nc.vector.tensor_add(
                            v_sb[:, ds(i * 4 + sub, 1), :, 0:DH],
                            big[:, sub * 512:(sub + 1) * 512].rearrange(
                                "p (o h e) -> p o h e", o=1, e=DH),
                            bv_sb[:].rearrange("p (o h e) -> p o h e",
                                               o=1, e=DH))

                    # ---- attention: dynamic pair loop, merged group loop --
                    with tc.For_i(0, NPAIR, 1) as pair:
                        big = bigp.tile([P, 4096], FP32, tag="big")
                        qg = kg_pool.tile([P, QTS], BF16, tag="qg")
                        nc.vector.tensor_copy(
                            qg[:].rearrange("p (o n) -> p o n", o=1),
                            qT[:, ds(pair, 1), ds(i * QTS, QTS)])
                        for h in range(2):  # clear at2 region
                            nc.tensor.matmul(
                                big[0:DH + 1,
                                    2048 + h * 512:2048 + (h + 1) * 512],
                                zrow[0:1, 0:DH + 1], zrow[0:1, :],
                                start=True, stop=True, skip_group_check=True)
                        with tc.For_i(0, 2 * i + 2, 1) as g:
                            ktg = kg_pool.tile([P, 256], BF16, tag="ktg")
                            vg = kg_pool.tile([P, 2, 2, DH + 1], BF16,
                                              tag="vg")
                            nc.vector.tensor_copy(
                                ktg[:].rearrange("p (o n) -> p o n", o=1),
                                kT[:, ds(pair, 1), ds(g * 256, 256)])
                            nc.vector.tensor_copy(
                                vg[:], v_sb[:, ds(g * 2, 2),
                                            ds(pair * 2, 2), :])
                            for h in range(2):
                                for t in range(2):
                                    nc.tensor.matmul(
                                        big[:, (2 * h + t) * 512:
                                            (2 * h + t + 1) * 512],
                                        ktg[h * DH:(h + 1) * DH,
                                            t * 128:(t + 1) * 128],
                                        qg[h * DH:(h + 1) * DH, :],
                                        start=True, stop=True)
                            # mask index: max(g-2i, -1)+1 -> 0 zeros,
                            # 1 m0, 2 m1 (additive -1e9 masks, pre-exp)
                            rv = nc.vector.alloc_register()
                            nc.vector.reg_alu(rv, g - 2 * i, -1, ALU.max)
                            nc.vector.reg_add(rv, rv, 1)
                            midx = nc.snap(rv, min_val=0, max_val=2)
                            nc.vector.tensor_add(
                                big[:, 0:2048].rearrange(
                                    "p (o n) -> p o n", o=1),
                                big[:, 0:2048].rearrange(
                                    "p (o n) -> p o n", o=1),
                                gm_sb[:, ds(midx, 1), :])
                            pr = pr_pool.tile([P, 2048], BF16, tag="pr")
                            nc.scalar.activation(pr[:], big[:, 0:2048],
                                                 AF.Exp,
                                                 scale=1.0 / np.sqrt(DH))
                            for h in range(2):
                                for t in range(2):
                                    nc.tensor.matmul(
                                        big[0:DH + 1, 2048 + h * 512:
                                            2048 + (h + 1) * 512],
                                        vg[:, t, h, :],
                                        pr[:, (2 * h + t) * 512:
                                           (2 * h + t + 1) * 512],
                                        start=False, stop=False,
                                        skip_group_check=True)
                        # ---- normalize ----
                        nc.vector.reciprocal(rr[0:1, :],
                                             big[DH:DH + 1, 2048:3072])
                        nc.tensor.matmul(big[0:DH, 3072:3584], ones64[0:1, :],
                                         rr[0:1, 0:512], start=True,
                                         stop=True)
                        nc.tensor.matmul(big[0:DH, 3584:4096], ones64[0:1, :],
                                         rr[0:1, 512:1024], start=True,
                                         stop=True)
                        rcs = kg_pool.tile([DH, 1024], BF16, tag="rcs")
                        nc.vector.tensor_copy(rcs[:], big[0:DH, 3072:4096])
                        aTg = kg_pool.tile([P, QTS], BF16, tag="aTg")
                        nc.vector.tensor_mul(aTg[0:DH, :],
                                             big[0:DH, 2048:2560],
                                             rcs[:, 0:512])
                        nc.vector.tensor_mul(aTg[DH:P, :],
                                             big[0:DH, 2560:3072],
                                             rcs[:, 512:1024])
                        nc.vector.tensor_copy(
                            aTn[:, ds(pair, 1), ds(i * QTS, QTS)],
                            aTg[:].rearrange("p (o n) -> p o n", o=1))

                    # ---- out projection: accumulate over dynamic pc ----
                    yt = yt_pool.tile([P, 8, QTS], BF16, tag="yt")
                    big = bigp.tile([P, 4096], FP32, tag="big")
                    for db in range(8):
                        nc.tensor.matmul(big[:, db * 512:(db + 1) * 512],
                                         zrow[0:1, 0:P], zrow[0:1, :],
                                         start=True, stop=True,
                                         skip_group_check=True)
                    with tc.For_i(0, NPAIR, 1) as pc:
                        wpcur = kg_pool.tile([P, D], BF16, tag="wpcur")
                        nc.vector.tensor_copy(
                            wpcur[:].rearrange("p (o n) -> p o n", o=1),
                            wp_sb[:, ds(pc, 1), :])
                        acur = kg_pool.tile([P, QTS], BF16, tag="acur")
                        nc.vector.tensor_copy(
                            acur[:].rearrange("p (o n) -> p o n", o=1),
                            aTn[:, ds(pc, 1), ds(i * QTS, QTS)])
                        for db in range(8):
                            nc.tensor.matmul(
                                big[:, db * 512:(db + 1) * 512],
                                wpcur[:, db * P:(db + 1) * P], acur[:],
                                start=False, stop=False,
                                skip_group_check=True)
                    for db in range(8):
                        nc.vector.tensor_scalar_add(
                            yt[:, db, :], big[:, db * 512:(db + 1) * 512],
                            bpT_sb[:, db:db + 1])
                    nc.sync.dma_start(
                        out=ypT[ds(i, 1), :, :].rearrange("o p n -> (o p) n"),
                        in_=yt[:].rearrange("p c t -> p (c t)"))

    nc.compile()
    return nc


_NC_CACHE = {}


def _get_nc():
    if "nc" not in _NC_CACHE:
        _NC_CACHE["nc"] = build_nc()
    return _NC_CACHE["nc"]


def make_in_maps(x, w_attn, b_attn, w_proj, b_proj):
    """Host-side sharding + layout prep: batch c//2, head-group c%2."""
    bf = ml_dtypes.bfloat16
    wq, wk, wvf = w_attn[:, :D], w_attn[:, D:2 * D], w_attn[:, 2 * D:]
    bq, bk, bvf = b_attn[:D], b_attn[D:2 * D], b_attn[2 * D:]
    in_maps = []
    for c in range(N_CORES):
        b, g = c // 2, c % 2
        sl = slice(g * VC, (g + 1) * VC)

        xT = np.ascontiguousarray(x[b].T.astype(bf))          # [1024, 2048]
        # xq[i, p, c*512+t] = xT[c*128+p, i*512+t]
        xq_np = np.ascontiguousarray(
            xT.reshape(KC, P, NQT, QTS).transpose(2, 1, 0, 3)
            .reshape(NQT, P, KC * QTS))

        def wlay(w):
            return np.ascontiguousarray(
                w.reshape(KC, P, w.shape[1]).transpose(1, 0, 2).astype(bf))

        wqk_np = wlay(np.concatenate([wq[:, sl], wk[:, sl]], axis=1))
        wv_np = wlay(wvf[:, sl])
        wp_np = np.ascontiguousarray(
            w_proj[g * VC:(g + 1) * VC, :].reshape(NPAIR, P, D)
            .transpose(1, 0, 2).astype(bf))

        bqk_np = np.ascontiguousarray(
            np.concatenate([bq[sl], bk[sl]]).reshape(8, P).T
            .astype(np.float32))
        bv_np = np.ascontiguousarray(
            np.broadcast_to(bvf[sl][None, :], (P, VC)).astype(np.float32))
        bpT_np = np.ascontiguousarray(
            (b_proj * 0.5).reshape(8, P).T.astype(np.float32))

        in_maps.append({"xq": xq_np, "wqk": wqk_np, "wv": wv_np, "wp": wp_np,
                        "bqk": bqk_np, "bv": bv_np, "bpT": bpT_np})
    return in_maps


def kernel(x, w_attn, b_attn, w_proj, b_proj):
    x = np.asarray(x, dtype=np.float32)
    w_attn = np.asarray(w_attn, dtype=np.float32)
    b_attn = np.asarray(b_attn, dtype=np.float32)
    w_proj = np.asarray(w_proj, dtype=np.float32)
    b_proj = np.asarray(b_proj, dtype=np.float32)

    nc = _get_nc()
    in_maps = make_in_maps(x, w_attn, b_attn, w_proj, b_proj)
    res = run_bass_kernel_spmd(nc, in_maps, core_ids=list(range(N_CORES)))
    out = np.empty((B, S, D), dtype=np.float32)
    for b in range(B):
        parts = []
        for c in (2 * b, 2 * b + 1):
            yp = np.asarray(res.results[c]["ypT"],
                            dtype=np.float32).reshape(NQT, P, 8, QTS)
            parts.append(yp.transpose(0, 3, 2, 1).reshape(S, D))
        out[b] = parts[0] + parts[1]
    return out
